# revision 1
# baseline (speedup 1.0000x reference)
"""AttentionBasedPruner Trainium2 kernel.

Per row (batch): scores = gelu(x @ w1 + b1) @ w2; top-k (k=2867 of 4096) by
threshold bisection; emit kept rows of x in ascending index order.

Sharding: batch 32 -> 8 cores x 4 rows (data parallel, no collectives).

Design (single pass over x, ~540 us/core vs ~1000 us for the two-pass
baseline):
  - x chunks stay RESIDENT in SBUF (13 rotating slots of [128, 4, 768]);
    pass 2 scatters straight from those tiles - no HBM re-read.
  - scores accumulate directly into a PSUM [128, 32] tile in the
    token = 128*c + p layout via per-128-token-group w2 matmuls (no DRAM
    round-trip reshape).
  - threshold search: 34 lo-only bisection steps (mid = lo + 2^-i * range;
    count(s >= mid) via a ones-matmul partition reduce), emitted INTERLEAVED
    into the next row's chunk compute so the serial dependency chain hides
    under PE work instead of head-of-line-blocking any queue.
  - indirect scatters round-robin over 4 output tensors: Tile chains
    same-tensor indirect DMAs on a completion semaphore (conservative WAW),
    so 4 targets give 4 concurrent chains through the Pool 4-deep wait
    queue; the host merges with a sum (disjoint rows, zeros elsewhere).
  - engine split: PE transposes+matmuls+count, ACT psum->sbuf copies+gelu,
    DVE bisect+slot math, Pool indirect scatters, SP loads.
"""
import sys

sys.path.insert(0, "/opt/trn_rl_repo")
import numpy as np

B, N, D, H = 32, 4096, 768, 192
KEEP = int(N * 0.7)  # 2867
NCORES = 8
RPC = B // NCORES  # rows per core
MC = N // 512  # 512-token chunks per row
BIG = 3.0e7
NBITS = 34
XSLOTS = 13

_cache = {}


def _build(stages=3, reps=1, batch_scatter=False, nbits=NBITS, f32r=False,
           w2big=False, sim_scatter_small=False, nsplit=4, finish_mc=2,
           hl3=False, csplit=True):
    key = ("nc", stages, reps, batch_scatter, nbits, f32r, w2big,
           sim_scatter_small, nsplit, finish_mc, hl3, csplit)
    if key in _cache:
        return _cache[key]
    import concourse.bacc as bacc
    import concourse.tile as tile
    import concourse.mybir as mybir
    import concourse.bass as bass
    from concourse.masks import make_identity

    F32 = mybir.dt.float32
    BF16 = mybir.dt.bfloat16
    F16 = mybir.dt.float16
    I32 = mybir.dt.int32
    U8 = mybir.dt.uint8
    MMDT = mybir.dt.float32r if f32r else F32
    GELU = mybir.ActivationFunctionType.Gelu
    COPY = mybir.ActivationFunctionType.Copy
    ALU = mybir.AluOpType
    AX = mybir.AxisListType.X

    nc = bacc.Bacc(None, target_bir_lowering=False)
    X = nc.dram_tensor("x", [RPC, N, D], F32, kind="ExternalInput")
    W1 = nc.dram_tensor("w1", [D, H], F32, kind="ExternalInput")
    B1 = nc.dram_tensor("b1", [H], F32, kind="ExternalInput")
    W2 = nc.dram_tensor("w2", [H, 1], F32, kind="ExternalInput")
    # round-robin scatter targets: Tile serializes indirect DMAs to the same
    # DRAM tensor on a completion-sem chain (conservative WAW); splitting the
    # columns across nsplit tensors gives nsplit concurrent chains. Each
    # output row is written by exactly one stream (rest stay zero); the host
    # merges with a sum.
    Ys = [
        nc.dram_tensor(f"y{i}", [RPC * KEEP, D], F32, kind="ExternalOutput")
        for i in range(nsplit)
    ]

    with tile.TileContext(nc) as tc:
        with (
            tc.tile_pool(name="const", bufs=1) as cp,
            tc.tile_pool(name="xin", bufs=XSLOTS) as xpool,
            tc.tile_pool(name="xt", bufs=2) as xtpool,
            tc.tile_pool(name="gt", bufs=2) as gtpool,
            tc.tile_pool(name="row", bufs=1) as rowpool,
            tc.tile_pool(name="bis", bufs=2) as bp,
            tc.tile_pool(name="psT", bufs=2, space="PSUM") as psT,
            tc.tile_pool(name="psH", bufs=1, space="PSUM") as psH,
            tc.tile_pool(name="psS", bufs=1, space="PSUM") as psSp,
            tc.tile_pool(name="psC", bufs=1, space="PSUM") as psC,
            tc.tile_pool(name="psW", bufs=1, space="PSUM") as psW,
        ):
            ident = cp.tile([128, 128], F32)
            make_identity(nc, ident[:])
            onesb = cp.tile([128, 128], BF16)
            nc.vector.memset(onesb[:], 1.0)
            onesf = cp.tile([128, 128], F32)
            nc.vector.memset(onesf[:], 1.0)
            # ltri[k, m] = 1 iff k <= m (inclusive prefix over partitions)
            ltri = cp.tile([128, 128], F32)
            nc.vector.memset(ltri[:], 1.0)
            nc.gpsimd.affine_select(
                out=ltri[:], in_=ltri[:], compare_op=ALU.is_ge, fill=0.0,
                base=0, pattern=[[1, 128]], channel_multiplier=-1,
            )
            # prefetch the first two x chunks before the (smaller) weight
            # loads so transpose work can start as early as possible
            xs_prefetch = []
            for mc0 in range(2):
                xsp = xpool.tile([128, 4, D], F32, tag="xs", name=f"xsp{mc0}")
                nc.sync.dma_start(
                    xsp[:],
                    X[0, mc0 * 512 : (mc0 + 1) * 512, :].rearrange(
                        "(g p) d -> p g d", p=128
                    ),
                )
                xs_prefetch.append(xsp)
            wdma = nc.gpsimd if f32r else nc.sync  # dtype-cast DMA needs SWDGE
            w1s = []
            for j in range(6):
                t = cp.tile([128, H], MMDT, tag=f"w1s{j}", name=f"w1s{j}")
                wdma.dma_start(t[:], W1[128 * j : 128 * (j + 1), :])
                w1s.append(t)
            w1hi, w1lo = [], []
            if hl3:
                # fp16 hi/lo split of w1: w1 = hi + lo to ~2^-22 relative;
                # x @ w1 then runs as 3 fp16 matmuls (xh*wh + xh*wl + xl*wh)
                # at 1 cyc/col instead of 1 fp32 matmul at 4 cyc/col.
                with nc.allow_low_precision(reason="fp16 hi/lo exact split"):
                    for j in range(6):
                        hi = cp.tile([128, H], F16, tag=f"w1hi{j}")
                        nc.vector.tensor_copy(hi[:], w1s[j][:])
                        hi32 = cp.tile([128, H], F32, tag=f"w1hi32{j}")
                        nc.vector.tensor_copy(hi32[:], hi[:])
                        lo = cp.tile([128, H], F16, tag=f"w1lo{j}")
                        nc.vector.tensor_sub(lo[:], w1s[j][:], hi32[:])
                        w1hi.append(hi)
                        w1lo.append(lo)
            b1a = cp.tile([128, 1], F32)
            nc.sync.dma_start(b1a[:], B1[0:128, None])
            b1b = cp.tile([64, 1], F32)
            nc.sync.dma_start(b1b[:], B1[128:H, None])
            w2a = cp.tile([128, 1], MMDT)
            wdma.dma_start(w2a[:], W2[0:128, :])
            w2b = cp.tile([64, 1], MMDT)
            wdma.dma_start(w2b[:], W2[128:H, :])

            pending = []

            def drain(k):
                for _ in range(min(k, len(pending))):
                    pending.pop(0)()

            def emit_chunk(r, mc, psS_r, rep=0):
                if rep == 0 and r == 0 and mc < len(xs_prefetch):
                    xs = xs_prefetch[mc]
                else:
                    xs = xpool.tile([128, 4, D], F32, tag="xs")
                    nc.sync.dma_start(
                        xs[:],
                        X[r, mc * 512 : (mc + 1) * 512, :].rearrange(
                            "(g p) d -> p g d", p=128
                        ),
                    )
                if hl3:
                    xh = [
                        xtpool.tile([128, 512], F16, tag=f"xh{j}", name=f"xh{j}") for j in range(6)
                    ]
                    xl = [
                        xtpool.tile([128, 512], F16, tag=f"xl{j}", name=f"xl{j}") for j in range(6)
                    ]
                else:
                    xts = [
                        xtpool.tile([128, 512], MMDT, tag=f"xt{j}", name=f"xts{j}")
                        for j in range(6)
                    ]
                for j in range(6):
                    pt = psT.tile([128, 512], F32, tag="tr")
                    for g in range(4):
                        nc.tensor.transpose(
                            pt[:, 128 * g : 128 * (g + 1)],
                            xs[:, g, 128 * j : 128 * (j + 1)],
                            ident[:],
                        )
                        if g == 1:
                            drain(1)
                    drain(1)
                    if hl3:
                        nc.scalar.activation(xh[j][:], pt[:], COPY)
                        with nc.allow_low_precision(reason="fp16 hi/lo split"):
                            nc.vector.tensor_sub(xl[j][:], pt[:], xh[j][:])
                    elif csplit and j % 2 == 1:
                        nc.vector.tensor_copy(xts[j][:], pt[:])
                        drain(1)
                    else:
                        nc.scalar.activation(xts[j][:], pt[:], COPY)
                    drain(1)
                ph = psH.tile([128, 1024], F32, tag="ph")
                if hl3:
                    terms = [(w1hi, xh), (w1lo, xh), (w1hi, xl)]
                    nmm = 6 * len(terms)
                    k = 0
                    for ws, xv in terms:
                        for j in range(6):
                            nc.tensor.matmul(
                                ph[:, 0:512], ws[j][:, 0:128], xv[j][:],
                                start=(k == 0), stop=(k == nmm - 1),
                            )
                            k += 1
                        drain(1)
                    k = 0
                    for ws, xv in terms:
                        for j in range(6):
                            nc.tensor.matmul(
                                ph[0:64, 512:1024], ws[j][:, 128:H], xv[j][:],
                                start=(k == 0), stop=(k == nmm - 1),
                            )
                            k += 1
                        drain(1)
                else:
                    for j in range(6):
                        nc.tensor.matmul(
                            ph[:, 0:512], w1s[j][:, 0:128], xts[j][:],
                            start=(j == 0), stop=(j == 5),
                        )
                    drain(1)
                    for j in range(6):
                        nc.tensor.matmul(
                            ph[0:64, 512:1024], w1s[j][:, 128:H], xts[j][:],
                            start=(j == 0), stop=(j == 5),
                        )
                    drain(1)
                gt0 = gtpool.tile([128, 512], MMDT, tag="gt0")
                gt1 = gtpool.tile([64, 512], MMDT, tag="gt1")
                nc.scalar.activation(gt0[:], ph[:, 0:512], GELU, bias=b1a[:, 0:1])
                nc.scalar.activation(
                    gt1[:], ph[0:64, 512:1024], GELU, bias=b1b[:, 0:1]
                )
                drain(1)
                if w2big:
                    # one [1, 512] score row per chunk, then reshape into the
                    # psS [128, 32] layout via 4 PE transposes of [1, 128]
                    ps_s = psW.tile([1, 512], F32, tag="srow")
                    nc.tensor.matmul(ps_s[:], w2a[:], gt0[:], start=True, stop=False)
                    nc.tensor.matmul(ps_s[:], w2b[:], gt1[:], start=False, stop=True)
                    drain(1)
                    srow = bp.tile([1, 512], F32, tag="srow_sb")
                    nc.scalar.activation(srow[:], ps_s[:], COPY)
                    drain(1)
                    for g in range(4):
                        c = 4 * mc + g
                        nc.tensor.transpose(
                            psS_r[:, c : c + 1],
                            srow[0:1, 128 * g : 128 * (g + 1)],
                            ident[0:1, 0:1],
                        )
                    drain(1)
                else:
                    for g in range(4):
                        c = 4 * mc + g
                        nc.tensor.matmul(
                            psS_r[:, c : c + 1], gt0[:, 128 * g : 128 * (g + 1)],
                            w2a[:], start=True, stop=False,
                        )
                        nc.tensor.matmul(
                            psS_r[:, c : c + 1], gt1[:, 128 * g : 128 * (g + 1)],
                            w2b[:], start=False, stop=True,
                        )
                        drain(1)
                return xs

            def make_bisect(s2d, lo):
                steps = []
                for i in range(nbits):
                    half = 64.0 / float(2 ** (i + 1))

                    def step(half=half):
                        mid = bp.tile([128, 1], F32, tag="mid")
                        nc.vector.tensor_scalar_add(mid[:], lo[:], half)
                        cmp = bp.tile([128, 32], BF16, tag="cmp")
                        nc.vector.tensor_scalar(
                            cmp[:], s2d[:], mid[:, 0:1], None, op0=ALU.is_ge
                        )
                        part = bp.tile([128, 1], BF16, tag="part")
                        # counts <= 32 are exact in bf16; bf16 operands let the
                        # count matmul use fast weight loads
                        with nc.allow_low_precision(reason="int counts <= 32"):
                            nc.vector.reduce_sum(part[:], cmp[:], axis=AX)
                        pcnt = psC.tile([128, 1], F32, tag="cnt")
                        nc.tensor.matmul(
                            pcnt[:], onesb[:], part[:], start=True, stop=True
                        )
                        ge = bp.tile([128, 1], U8, tag="ge")
                        nc.vector.tensor_scalar(
                            ge[:], pcnt[:], float(KEEP), None, op0=ALU.is_ge
                        )
                        nc.vector.copy_predicated(lo[:], ge[:], mid[:])

                    steps.append(step)
                return steps

            def emit_finish_row(r, s2d, lo, xs_list):
                kept = rowpool.tile([128, 32], F32, tag=f"kept{r}")
                nc.vector.tensor_scalar(
                    kept[:], s2d[:], lo[:, 0:1], None, op0=ALU.is_ge
                )
                ic = psC.tile([128, 64], F32, tag="ic")
                pincl = ic[:, 0:32]
                pcols = ic[:, 32:64]
                nc.tensor.matmul(pincl, ltri[:], kept[:], start=True, stop=True)
                nc.tensor.matmul(pcols, onesf[:], kept[:], start=True, stop=True)
                exA = bp.tile([128, 32], F32, tag="exA")
                nc.vector.tensor_sub(exA[:], pincl, kept[:])
                cur = bp.tile([128, 32], F32, tag="scan0")
                nc.vector.memset(cur[:, 0:1], 0.0)
                nc.vector.tensor_copy(cur[:, 1:32], pcols[:, 0:31])
                for i, d in enumerate((1, 2, 4, 8, 16)):
                    nxt = bp.tile([128, 32], F32, tag=f"scan{1 - i % 2}")
                    nc.vector.tensor_copy(nxt[:, 0:d], cur[:, 0:d])
                    nc.vector.tensor_add(
                        nxt[:, d:32], cur[:, d:32], cur[:, 0 : 32 - d]
                    )
                    cur = nxt
                slot = bp.tile([128, 32], F32, tag="slot")
                nc.vector.tensor_add(slot[:], exA[:], cur[:])
                nc.vector.tensor_scalar_add(slot[:], slot[:], float(r * KEEP))
                keptu = bp.tile([128, 32], U8, tag="keptu")
                nc.vector.tensor_copy(keptu[:], kept[:])
                slotf = bp.tile([128, 32], F32, tag="slotf")
                nc.vector.memset(slotf[:], BIG)
                nc.vector.copy_predicated(slotf[:], keptu[:], slot[:])
                didx = rowpool.tile([128, 32], I32, tag=f"didx{r}")
                nc.vector.tensor_copy(didx[:], slotf[:])
                # sim_scatter_small: cost-model-only build — shrink the
                # scatter's out AP so TimelineSim prices descriptors from
                # the actually-written 128 rows, not all of Y (semantics
                # of this build are wrong; never run it on HW)
                for c in range(32):
                    mc, g = divmod(c, 4)
                    Yt = Ys[c % nsplit]
                    yout = Yt[0:128, :] if sim_scatter_small else Yt[:, :]
                    nc.gpsimd.indirect_dma_start(
                        out=yout,
                        out_offset=bass.IndirectOffsetOnAxis(
                            ap=didx[:, c : c + 1], axis=0
                        ),
                        in_=xs_list[mc][:, g, :],
                        in_offset=None,
                        bounds_check=(r + 1) * KEEP - 1,
                        oob_is_err=False,
                    )

            prev = None
            for rep in range(reps):
                for r in range(RPC):
                    psS_r = psSp.tile([128, 32], F32, tag="psS")
                    xs_list = []
                    for mc in range(MC):
                        if prev is not None and mc == finish_mc and stages >= 3:
                            drain(len(pending))
                            emit_finish_row(*prev)
                            prev = None
                        xs_list.append(emit_chunk(r, mc, psS_r, rep))
                    s2d = rowpool.tile([128, 32], F32, tag=f"s2d{rep}_{r}")
                    nc.vector.tensor_copy(s2d[:], psS_r[:])
                    if stages < 2:
                        continue
                    lo = rowpool.tile([128, 1], F32, tag=f"lo{rep}_{r}")
                    nc.vector.memset(lo[:], -32.0)
                    pending.extend(make_bisect(s2d, lo))
                    prev = (r, s2d, lo, xs_list)
                drain(len(pending))
                if prev is not None and stages >= 3:
                    emit_finish_row(*prev)
                    prev = None

    nc.compile()
    _cache[key] = nc
    return nc


def kernel(x, w1, b1, w2, b2=None, trace=False, stages=3, **build_kw):
    from concourse.bass_utils import run_bass_kernel_spmd

    nc = _build(stages, **build_kw)
    x = np.ascontiguousarray(np.asarray(x, dtype=np.float32))
    w1 = np.ascontiguousarray(np.asarray(w1, dtype=np.float32))
    b1 = np.ascontiguousarray(np.asarray(b1, dtype=np.float32))
    w2 = np.ascontiguousarray(np.asarray(w2, dtype=np.float32))
    in_maps = [
        {
            "x": np.ascontiguousarray(x[c * RPC : (c + 1) * RPC]),
            "w1": w1,
            "b1": b1,
            "w2": w2,
        }
        for c in range(NCORES)
    ]
    res = run_bass_kernel_spmd(
        nc, in_maps, core_ids=list(range(NCORES)), trace=trace
    )

    def merge(rc):
        if "y" in rc:
            return rc["y"]
        acc = rc["y0"].copy()
        i = 1
        while f"y{i}" in rc:
            acc += rc[f"y{i}"]
            i += 1
        return acc

    out = np.concatenate(
        [merge(res.results[c]).reshape(RPC, KEEP, D) for c in range(NCORES)],
        axis=0,
    )
    if trace:
        return out, res
    return out

